# revision 23
# baseline (speedup 1.0000x reference)
"""DualPathAttention Trainium2 Bass kernel (pipelined, bf16; ~372us vs 578us baseline).

Sharding: batch*head parallel across 8 cores. Core c handles batch b=c//4 and
global heads [4*(c%4), 4*(c%4)+4). Each core computes its 4 heads' dual-path
attention and the partial final projection (its 256 rows of out_w); the host
sums the 4 partials per batch and adds out_b.

Structure (all matmuls bf16, PSUM f32, line norm math f32):
  Prologue: Pluecker line operands projected with (wla|wlc) / (wlb|wld)
    packed into M=128 stationaries (kc-outer over all 4 q-blocks so the PE
    chews chunks as the x DMA stream lands). The A*B - C*D subtraction runs
    on the PE via a +/-1 stationary (the two products live in opposite SBUF
    quadrants; DVE lanes cannot cross). Gate rows + sigmoid batched here so
    the ACT table never leaves exp in the main loop. x_prev shift via a
    host-padded leading zero column in xT, read one column left from the
    staged projections.
  Main loop per q-block tb: q/k/v/gv projections (own PSUM pool so they
    never wait on attention accumulators), then attention j=tb (ACT-heavy
    exps overlap the next block's projections), then the final projection of
    block tb. Softmax denominator rides at row 0 of the AV matmul (ones in
    col 0 of each v group) so the reciprocal runs at partition 0 — no
    partition-shift DMAs. Diagonal k-tiles narrowed to cols >= 128*m.
"""

import os
import numpy as np
import ml_dtypes

import concourse.bass as bass
from concourse import bacc
import concourse.mybir as mybir
import concourse.tile as tile
from concourse.bass_utils import run_bass_kernel_spmd

D, H, B, T = 1024, 16, 2, 2048
DH = 64          # head dim
NH = 4           # heads per core
NCORES = 8
QB = 512         # q block width
KT = 128         # k tile height
NQB = T // QB    # 4
NKC = 8          # 128-deep chunks of D
F32 = mybir.dt.float32
BF16 = mybir.dt.bfloat16

PAIRS4 = [(0, 1), (0, 2), (0, 3), (1, 2), (1, 3), (2, 3)]
SIGMA = [1.0, -1.0, 1.0, 1.0, -1.0, 1.0]

TRACE = False            # set by test harness for profiling runs
DEBUG = False
LAST_RESULT = None       # BassKernelResults of last run (for exec_time_ns)


def _build_nc():
    nc = bacc.Bacc("TRN2", target_bir_lowering=False, debug=False)

    # ---- DRAM I/O (host pre-shuffles everything into SBUF layouts) ----
    d_xT = nc.dram_tensor("xT", [128, NKC * (T + 1)], BF16, kind="ExternalInput")
    d_wlac = nc.dram_tensor("wlac", [128, NKC * 128], BF16, kind="ExternalInput")
    d_wlbd = nc.dram_tensor("wlbd", [128, NKC * 128], BF16, kind="ExternalInput")
    d_seln = nc.dram_tensor("seln", [128, 64], BF16, kind="ExternalInput")
    d_wq = nc.dram_tensor("wq", [128, NKC * 256], BF16, kind="ExternalInput")
    d_wk = nc.dram_tensor("wk", [128, NKC * 256], BF16, kind="ExternalInput")
    d_wvg = nc.dram_tensor("wvg", [128, NKC * 512], BF16, kind="ExternalInput")
    d_wgate = nc.dram_tensor("wgate", [128, NKC * 16], BF16, kind="ExternalInput")
    d_outw = nc.dram_tensor("outw", [128, 2 * D], BF16, kind="ExternalInput")
    d_ssel = nc.dram_tensor("ssel", [128, 64], BF16, kind="ExternalInput")
    d_bq = nc.dram_tensor("bq", [128, 2], F32, kind="ExternalInput")
    d_bk = nc.dram_tensor("bk", [128, 2], F32, kind="ExternalInput")
    d_bvg = nc.dram_tensor("bvg", [1, 512], BF16, kind="ExternalInput")
    d_bgate = nc.dram_tensor("bgate", [16, 1], F32, kind="ExternalInput")
    d_sbc = nc.dram_tensor("sbc", [64, 1], F32, kind="ExternalInput")
    d_partial = nc.dram_tensor("partial", [T, D], F32, kind="ExternalOutput")

    AF = mybir.ActivationFunctionType
    OP = mybir.AluOpType

    with tile.TileContext(nc, linearize=bool(int(os.environ.get('KLIN', '0')))) as tc:
        with (
            tc.tile_pool(name="const", bufs=1) as cpool,
            tc.tile_pool(name="pers", bufs=1) as pers,
        ):
            # ---- persistent tiles ----
            xT_sb = cpool.tile([128, NKC, T + 1], BF16)
            wlac_sb = cpool.tile([128, NKC, 128], BF16)
            wlbd_sb = cpool.tile([128, NKC, 128], BF16)
            seln_sb = cpool.tile([128, 64], BF16)
            wq_sb = cpool.tile([128, NKC, 256], BF16)
            wk_sb = cpool.tile([128, NKC, 256], BF16)
            wvg_sb = cpool.tile([128, NKC, 512], BF16)
            wgate_sb = cpool.tile([128, NKC, 16], BF16)
            outw_sb = cpool.tile([128, 2, D], BF16)
            ssel_sb = cpool.tile([128, 64], BF16)
            bq_sb = cpool.tile([128, 2], F32)
            bk_sb = cpool.tile([128, 2], F32)
            bvg_sb = cpool.tile([1, 512], BF16)
            bgate_sb = cpool.tile([16, 1], F32)
            sbc_sb = cpool.tile([64, 1], F32)
            ones_bf = cpool.tile([1, 128], BF16)
            gsel = cpool.tile([16, 1], BF16)

            qT = pers.tile([128, 2, T], BF16)
            kTp = pers.tile([128, NH, T], BF16)   # head h: rows [64*(h%2),+64)
            vplus = pers.tile([128, 16, NH * 65], BF16)  # group col0 = ones
            gvplus = pers.tile([128, 16, NH * 65], BF16)
            jwT = pers.tile([128, T], BF16)       # head h at rows [32h, 32h+6)
            rlT = pers.tile([128, T], BF16)
            ggc = pers.tile([1, NQB, 2 * QB], F32)  # per block: [1-g | g]
            # head-pair packed: pair hp rows 0:64 = head 2hp, 64:128 = 2hp+1
            comb = pers.tile([128, 2, 2, QB], BF16)

            # ---- DMA issue order: lines weights, x stream, the rest ----
            nc.sync.dma_start(wlac_sb[:], d_wlac[:])
            nc.sync.dma_start(wlbd_sb[:], d_wlbd[:])
            for kc in range(NKC):
                nc.sync.dma_start(xT_sb[:, kc, :],
                                  d_xT[:, (T + 1) * kc:(T + 1) * (kc + 1)])
                if kc == 0:
                    nc.sync.dma_start(seln_sb[:], d_seln[:])
                    nc.sync.dma_start(ssel_sb[:], d_ssel[:])
                    nc.sync.dma_start(sbc_sb[:], d_sbc[:])
                    nc.sync.dma_start(bgate_sb[:], d_bgate[:])
                    nc.sync.dma_start(bq_sb[:], d_bq[:])
                    nc.sync.dma_start(bk_sb[:], d_bk[:])
                    nc.sync.dma_start(bvg_sb[:], d_bvg[:])
            nc.sync.dma_start(wgate_sb[:], d_wgate[:])
            nc.sync.dma_start(wq_sb[:], d_wq[:])
            nc.sync.dma_start(wk_sb[:], d_wk[:])
            nc.sync.dma_start(wvg_sb[:], d_wvg[:])
            nc.sync.dma_start(outw_sb[:], d_outw[:])

            # ---- constant fills (gpsimd; off every critical path at t0) ----
            nc.gpsimd.memset(ones_bf[:], 1.0)
            nc.gpsimd.memset(gsel[:], 1.0 / 16.0)
            for hh in range(NH):
                nc.gpsimd.memset(kTp[64 * ((hh + 1) % 2):64 * ((hh + 1) % 2) + 64,
                                     hh, :], 0.0)
            for vp in (vplus, gvplus):
                nc.gpsimd.memset(
                    vp[:].rearrange("p t (h c) -> p t h c", c=65)[:, :, :, 0:1],
                    1.0)

            # ================= Prologue: Pluecker lines + gate ===========
            # Packed operands (128 x T): rows 0:64 = A-side (wla: write rows
            # 0:24+pad, read rows 32:56+pad), rows 64:128 = C-side (wlc).
            # B/D sides likewise in wlbd. lines_u = A*B - C*D via the +/-1
            # stationary seln. Write rows read the staged projection shifted
            # one column left (x_prev); read rows unshifted.
            with (
                tc.tile_pool(name="lines", bufs=1) as lnp,
                tc.tile_pool(name="lnsub", bufs=4) as lns,
                tc.tile_pool(name="psPro", bufs=8,
                             space=bass.MemorySpace.PSUM) as psp,
            ):
                stg = lnp.tile([128, T + 1], F32)   # staged AC projections
                t1 = lnp.tile([64, T], F32)         # lines_u
                sq = lnp.tile([128, T], BF16)       # squares (K-padded)
                nc.gpsimd.memset(stg[:, 0:1], 0.0)
                nc.gpsimd.memset(sq[64:128, :], 0.0)

                pxs = [psp.tile([128, QB], F32, tag="pro", name=f"px{t}")
                       for t in range(NQB)]
                pys = [psp.tile([128, QB], F32, tag="pro", name=f"py{t}")
                       for t in range(NQB)]
                for kc in range(NKC):
                    for tb in range(NQB):
                        sl = slice(1 + QB * tb, 1 + QB * (tb + 1))
                        nc.tensor.matmul(pxs[tb][:], wlac_sb[:, kc, :],
                                         xT_sb[:, kc, sl],
                                         start=(kc == 0), stop=(kc == 7))
                        nc.tensor.matmul(pys[tb][:], wlbd_sb[:, kc, :],
                                         xT_sb[:, kc, sl],
                                         start=(kc == 0), stop=(kc == 7))

                psUl = []
                for tb in range(NQB):
                    gsl = slice(QB * tb, QB * (tb + 1))
                    ssl = slice(1 + QB * tb, 1 + QB * (tb + 1))
                    nc.scalar.copy(stg[:, ssl], pxs[tb][:])
                    prod = lns.tile([128, QB], BF16, tag="pr")
                    # rows 0:64 = write components (x_prev shift), 64:128 =
                    # read components (unshifted)
                    nc.vector.tensor_mul(prod[0:64, :],
                                         stg[0:64, gsl], pys[tb][0:64, :])
                    nc.vector.tensor_mul(prod[64:128, :],
                                         stg[64:128, ssl], pys[tb][64:128, :])
                    psU = psp.tile([64, QB], F32, tag="pro", name=f"pu{tb}")
                    nc.tensor.matmul(psU[:], seln_sb[:], prod[:],
                                     start=True, stop=True)
                    psUl.append(psU)
                    nc.scalar.copy(t1[:, gsl], psU[:])
                    nc.scalar.square(sq[0:64, gsl], psU[:])

                # ---- gate rows (sigmoids batched: one table swap) ----
                gpss = []
                for tb in range(NQB):
                    sl = slice(1 + QB * tb, 1 + QB * (tb + 1))
                    gps = psp.tile([16, QB], F32, tag="pro", name=f"gp{tb}")
                    for kc in range(NKC):
                        nc.tensor.matmul(gps[:], wgate_sb[:, kc, :],
                                         xT_sb[:, kc, sl],
                                         start=(kc == 0), stop=(kc == 7))
                    gpss.append(gps)
                gsigs = []
                for tb in range(NQB):
                    gsig = lns.tile([16, QB], BF16, tag="gs", name=f"gs{tb}")
                    nc.scalar.activation(out=gsig[:], in_=gpss[tb][:],
                                         func=AF.Sigmoid,
                                         bias=bgate_sb[:, 0:1], scale=1.0)
                    gsigs.append(gsig)
                for tb in range(NQB):
                    gsl = slice(QB * tb, QB * (tb + 1))
                    psm = psp.tile([1, QB], F32, tag="pro")
                    nc.tensor.matmul(psm[:], gsel[:], gsigs[tb][:],
                                     start=True, stop=True)
                    nc.vector.tensor_copy(ggc[:, tb, QB:2 * QB], psm[:])
                    nc.vector.tensor_scalar(
                        out=ggc[:, tb, 0:QB], in0=psm[:],
                        scalar1=-1.0, scalar2=1.0, op0=OP.mult, op1=OP.add)

                # ---- norm: ssq matmuls, then sqrts batched ----
                ssqs, rts = [], []
                for tb in range(NQB):
                    gsl = slice(QB * tb, QB * (tb + 1))
                    ps = psp.tile([64, QB], F32, tag="pro")
                    nc.tensor.matmul(ps[:], ssel_sb[:], sq[:, gsl],
                                     start=True, stop=True)
                    ssq = lns.tile([64, QB], F32, tag="n1", name=f"ssq{tb}")
                    nc.vector.tensor_scalar_max(out=ssq[:], in0=ps[:],
                                                scalar1=1e-24)
                    ssqs.append(ssq)
                for tb in range(NQB):
                    rt = lns.tile([64, QB], F32, tag="n2", name=f"rt{tb}")
                    nc.scalar.sqrt(rt[:], ssqs[tb][:])
                    rts.append(rt)
                for tb in range(NQB):
                    gsl = slice(QB * tb, QB * (tb + 1))
                    inv = lns.tile([64, QB], F32, tag="n1")
                    nc.vector.reciprocal_approx_fast(out=inv[:], in_=rts[tb][:])
                    # fold inc_scale into read-line norms (rows 0:32 are 1.0)
                    nc.vector.tensor_scalar_mul(out=inv[:], in0=inv[:],
                                                scalar1=sbc_sb[:, 0:1])
                    t1b = lns.tile([64, QB], BF16, tag="n2")
                    nc.vector.tensor_mul(t1b[:], t1[:, gsl], inv[:])
                    for h in range(NH):
                        nc.sync.dma_start(out=jwT[32 * h:32 * h + 6, gsl],
                                          in_=t1b[6 * h:6 * h + 6, :])
                        nc.sync.dma_start(out=rlT[32 * h:32 * h + 6, gsl],
                                          in_=t1b[32 + 6 * h:32 + 6 * h + 6, :])

            # ================= Main loop: per q-block tb =================
            with (
                tc.tile_pool(name="psA", bufs=2,
                             space=bass.MemorySpace.PSUM) as psA_,
                tc.tile_pool(name="psU", bufs=2,
                             space=bass.MemorySpace.PSUM) as psU_,
                tc.tile_pool(name="psL", bufs=2,
                             space=bass.MemorySpace.PSUM) as psL,
                tc.tile_pool(name="ptile", bufs=4) as pp,
                tc.tile_pool(name="rows", bufs=3) as rowp,
                tc.tile_pool(name="uwork", bufs=2) as uwp,
                tc.tile_pool(name="outp", bufs=2) as otp,
            ):
                for tb in range(NQB):
                    gsl = slice(QB * tb, QB * (tb + 1))
                    xsl = slice(1 + QB * tb, 1 + QB * (tb + 1))

                    # ---- A2a: q / k for this block ----
                    for mc in range(2):
                        for (wsb, bias, isq) in ((wq_sb, bq_sb, True),
                                                 (wk_sb, bk_sb, False)):
                            ps = psA_.tile([128, QB], F32, tag="a")
                            for kc in range(NKC):
                                nc.tensor.matmul(
                                    ps[:], wsb[:, kc, 128 * mc:128 * (mc + 1)],
                                    xT_sb[:, kc, xsl],
                                    start=(kc == 0), stop=(kc == 7))
                            if isq:
                                nc.vector.tensor_scalar_add(
                                    out=qT[:, mc, gsl], in0=ps[:],
                                    scalar1=bias[:, mc:mc + 1])
                            else:
                                nc.vector.tensor_scalar_add(
                                    out=kTp[0:64, 2 * mc, gsl],
                                    in0=ps[0:64, :],
                                    scalar1=bias[0:64, mc:mc + 1])
                                nc.vector.tensor_scalar_add(
                                    out=kTp[64:128, 2 * mc + 1, gsl],
                                    in0=ps[64:128, :],
                                    scalar1=bias[64:128, mc:mc + 1])

                    # ---- A2b: v|gv stacked ----
                    for ti in range(4 * tb, 4 * tb + 4):
                        ps = psA_.tile([128, 512], F32, tag="a")
                        nc.tensor.matmul(ps[:], ones_bf[0:1, :], bvg_sb[:],
                                         start=True, stop=False)
                        for kc in range(NKC):
                            nc.tensor.matmul(
                                ps[:],
                                xT_sb[:, kc, 1 + 128 * ti:1 + 128 * (ti + 1)],
                                wvg_sb[:, kc, :],
                                start=False, stop=(kc == 7))
                        nc.vector.tensor_copy(
                            vplus[:, ti, :].rearrange(
                                "p (h c) -> p h c", c=65)[:, :, 1:65],
                            ps[:, 0:256].rearrange("p (h c) -> p h c", c=64))
                        nc.vector.tensor_copy(
                            gvplus[:, ti, :].rearrange(
                                "p (h c) -> p h c", c=65)[:, :, 1:65],
                            ps[:, 256:512].rearrange("p (h c) -> p h c", c=64))

                    # ---- B: dual-path attention, q-block j = tb ----
                    j = tb
                    j2 = j % 2
                    nkt = 4 * (j + 1)
                    for p in range(NH):
                        Us = psU_.tile([65, QB], F32, tag="u")
                        Ug = psU_.tile([65, QB], F32, tag="u")
                        for kt in range(nkt):
                            m = kt - 4 * j
                            c0 = KT * m if m > 0 else 0
                            ksl = slice(KT * kt, KT * (kt + 1))
                            LB = psL.tile([128, 2, QB], F32, tag="L")
                            nc.tensor.matmul(
                                LB[:, 0, c0:QB], kTp[:, p, ksl],
                                qT[:, p // 2, QB * j + c0:QB * (j + 1)],
                                start=True, stop=True)
                            nc.tensor.matmul(
                                LB[:, 1, c0:QB], jwT[32 * p:32 * p + 6, ksl],
                                rlT[32 * p:32 * p + 6, QB * j + c0:QB * (j + 1)],
                                start=True, stop=True,
                                tile_position=(32 * p, 0))
                            P = pp.tile([128, 2, QB], BF16, tag="P")
                            nc.scalar.activation(P[:, :, c0:QB],
                                                 LB[:, :, c0:QB], AF.Exp)
                            if m >= 0:
                                for path in range(2):
                                    # keep where (col-c0) - chan >= 0
                                    nc.gpsimd.affine_select(
                                        out=P[:, path, c0:c0 + KT],
                                        in_=P[:, path, c0:c0 + KT],
                                        compare_op=OP.is_ge, fill=0.0,
                                        base=0, pattern=[[1, KT]],
                                        channel_multiplier=-1)
                            nc.tensor.matmul(
                                Us[:, c0:QB],
                                vplus[:, kt, 65 * p:65 * p + 65],
                                P[:, 0, c0:QB],
                                start=(kt == 0), stop=(kt == nkt - 1))
                            nc.tensor.matmul(
                                Ug[:, c0:QB],
                                gvplus[:, kt, 65 * p:65 * p + 65],
                                P[:, 1, c0:QB],
                                start=(kt == 0), stop=(kt == nkt - 1))
                        # combine: alpha=(1-g)/Dstd, beta=g/Dgeo; denominators
                        # at PSUM partition 0 (ones col 0 of the v groups).
                        # Stage Us/Ug to SBUF first so the PSUM ring frees
                        # ~1.4us after the AV stop; the chain (recip, gate
                        # mul, gpsimd broadcast) then runs off the PE path.
                        d0e0 = rowp.tile([1, 2 * QB], F32, tag="r0")
                        nc.vector.tensor_copy(d0e0[:, 0:QB], Us[0:1, :])
                        nc.vector.tensor_copy(d0e0[:, QB:2 * QB], Ug[0:1, :])
                        UsS = uwp.tile([65, 2, QB], BF16, tag="uss")
                        nc.vector.tensor_copy(UsS[:, 0, :], Us[:])
                        nc.vector.tensor_copy(UsS[:, 1, :], Ug[:])
                        rsg = rowp.tile([1, 2 * QB], F32, tag="r1")
                        nc.vector.reciprocal_approx_fast(
                            out=rsg[:], in_=d0e0[:])
                        arbr = rowp.tile([1, 2 * QB], BF16, tag="r2")
                        nc.vector.tensor_mul(arbr[:], rsg[:], ggc[:, tb, :])
                        abB = uwp.tile([65, 2 * QB], BF16, tag="ab")
                        nc.gpsimd.partition_broadcast(abB[:], arbr[:],
                                                      channels=65)
                        u1 = uwp.tile([65, QB], BF16, tag="u")
                        us = uwp.tile([65, QB], BF16, tag="us")
                        # row 0 = denom*alpha junk — dropped by the DMA below
                        nc.vector.tensor_mul(u1[:], UsS[:, 0, :],
                                             abB[:, 0:QB])
                        nc.vector.tensor_mul(us[:], UsS[:, 1, :],
                                             abB[:, QB:2 * QB])
                        nc.vector.tensor_add(us[:], u1[:], us[:])
                        # pack head pairs for phase C (DMA shifts partitions)
                        nc.sync.dma_start(
                            comb[64 * (p % 2):64 * (p % 2) + 64,
                                 j2, p // 2, :],
                            us[1:65, :])

                    # ---- C: final projection for q-block j ----
                    for qt in range(4):
                        for et in range(2):
                            ps = psU_.tile([128, QB], F32, tag="u")
                            for hp in range(2):
                                nc.tensor.matmul(
                                    ps[:],
                                    comb[:, j2, hp, 128 * qt:128 * (qt + 1)],
                                    outw_sb[:, hp, QB * et:QB * (et + 1)],
                                    start=(hp == 0), stop=(hp == 1))
                            ot = otp.tile([128, QB], F32, tag="o")
                            nc.vector.tensor_copy(ot[:], ps[:])
                            nc.sync.dma_start(
                                d_partial[QB * j + 128 * qt:
                                          QB * j + 128 * (qt + 1),
                                          QB * et:QB * (et + 1)],
                                ot[:])
    nc.compile()
    return nc


_nc_cache = None


def _get_nc():
    global _nc_cache
    if _nc_cache is None:
        _nc_cache = _build_nc()
    return _nc_cache


def _kc_layout(w):
    """[D, C] -> [128, NKC*C] with row (p, kc) = D-index kc*128+p."""
    Dd, C = w.shape
    return np.ascontiguousarray(
        w.reshape(NKC, 128, C).transpose(1, 0, 2).reshape(128, NKC * C))


def _prep_core_inputs(inputs, core):
    b = core // 4
    h0 = (core % 4) * 4
    f = np.float32
    bf = ml_dtypes.bfloat16
    qkv_w, qkv_b = inputs['qkv_w'], inputs['qkv_b']
    scale = DH ** -0.5
    s = slice(h0 * DH, h0 * DH + NH * DH)
    ac = np.ascontiguousarray

    # Line operand layout: 64 cols = [write(24)+pad8 | read(24)+pad8]; A/C
    # from w1 (shifted = x_prev side), B/D from the w2/read counterparts.
    # The J6 contraction is folded into the write gather (reversed pairs +
    # signs). A|C and B|D are stacked into 128-wide stationaries.
    WLA = np.zeros((D, 64), f); WLB = np.zeros((D, 64), f)
    WLC = np.zeros((D, 64), f); WLD = np.zeros((D, 64), f)
    w1w, w2w = inputs['w1_write'], inputs['w2_write']
    w1r, w2r = inputs['w1_read'], inputs['w2_read']
    for h in range(NH):
        gh = h0 + h
        for jj in range(6):
            i_, j_ = PAIRS4[5 - jj]
            WLA[:, 0 + h * 6 + jj] = w1w[:, gh * 4 + i_] * SIGMA[jj]    # A_w
            WLB[:, 0 + h * 6 + jj] = w2w[:, gh * 4 + j_]                # B_w
            WLC[:, 0 + h * 6 + jj] = w1w[:, gh * 4 + j_] * SIGMA[jj]    # C_w
            WLD[:, 0 + h * 6 + jj] = w2w[:, gh * 4 + i_]                # D_w
        for pp_ in range(6):
            i_, j_ = PAIRS4[pp_]
            WLA[:, 32 + h * 6 + pp_] = w1r[:, gh * 4 + i_]              # A_r
            WLB[:, 32 + h * 6 + pp_] = w2r[:, gh * 4 + j_]              # B_r
            WLC[:, 32 + h * 6 + pp_] = w1r[:, gh * 4 + j_]              # C_r
            WLD[:, 32 + h * 6 + pp_] = w2r[:, gh * 4 + i_]              # D_r

    # +/-1 stationary: write lines m<32: prod[m]-prod[m+32];
    # read lines m>=32: prod[m+32]-prod[m+64]
    seln = np.zeros((128, 64), f)
    m = np.arange(32)
    seln[m, m] = 1.0
    seln[m + 32, m] = -1.0
    seln[m + 64, m + 32] = 1.0
    seln[m + 96, m + 32] = -1.0

    ssel = np.zeros((128, 64), f)
    for half in (0, 32):
        for h in range(NH):
            g = slice(half + 6 * h, half + 6 * h + 6)
            ssel[g, g] = 1.0
    sbc = np.ones((64, 1), f)
    sbc[32:56, 0] = np.repeat(inputs['inc_scale'][h0:h0 + NH], 6).astype(f)

    # x^T with a leading zero column per kc chunk (x_prev shift support)
    xT = np.asarray(inputs['x'][b], f).T            # [D, T]
    xTp = np.zeros((NKC, 128, T + 1), f)
    xTp[:, :, 1:] = xT.reshape(NKC, 128, T)
    xTp = xTp.transpose(1, 0, 2).reshape(128, NKC * (T + 1))

    # out_w head-pair packed: pair hp rows 0:64 = head 2hp, 64:128 = 2hp+1
    outw = np.zeros((128, 2, D), f)
    for hc in range(NH):
        outw[64 * (hc % 2):64 * (hc % 2) + 64, hc // 2, :] = \
            inputs['out_w'][(h0 + hc) * DH:(h0 + hc + 1) * DH, :]

    wv = qkv_w[:, 2 * D:3 * D][:, s].astype(f)
    wgv = inputs['geo_w'][:, s].astype(f)

    return {
        'xT': ac(xTp).astype(bf),
        'wlac': _kc_layout(np.concatenate(
            [WLA[:, 0:32], WLC[:, 0:32], WLA[:, 32:64], WLC[:, 32:64]],
            axis=1)).astype(bf),
        'wlbd': _kc_layout(np.concatenate(
            [WLB[:, 0:32], WLD[:, 0:32], WLB[:, 32:64], WLD[:, 32:64]],
            axis=1)).astype(bf),
        'seln': seln.astype(bf),
        'wq': _kc_layout((qkv_w[:, 0 * D:1 * D][:, s] * scale).astype(f)).astype(bf),
        'wk': _kc_layout(qkv_w[:, 1 * D:2 * D][:, s].astype(f)).astype(bf),
        'wvg': _kc_layout(np.concatenate([wv, wgv], axis=1)).astype(bf),
        'wgate': _kc_layout(inputs['gate_w'].astype(f)).astype(bf),
        'outw': ac(outw.reshape(128, 2 * D)).astype(bf),
        'ssel': ssel.astype(bf),
        'bq': ac((qkv_b[0 * D:1 * D][s] * scale).astype(f)
                 .reshape(2, 128).transpose(1, 0)),
        'bk': ac(qkv_b[1 * D:2 * D][s].astype(f).reshape(2, 128).transpose(1, 0)),
        'bvg': ac(np.concatenate(
            [qkv_b[2 * D:3 * D][s], inputs['geo_b'][s]]).astype(f)
            .reshape(1, 512)).astype(bf),
        'bgate': ac(inputs['gate_b'].astype(f).reshape(16, 1)),
        'sbc': sbc,
    }


def kernel(**inputs):
    global LAST_RESULT
    inputs = {k: np.asarray(v) for k, v in inputs.items()}
    nc = _get_nc()
    in_maps = [_prep_core_inputs(inputs, c) for c in range(NCORES)]
    res = run_bass_kernel_spmd(nc, in_maps, core_ids=list(range(NCORES)),
                               trace=TRACE)
    LAST_RESULT = res
    out = np.zeros((B, T, D), np.float32)
    for c in range(NCORES):
        out[c // 4] += res.results[c]['partial']
    out += np.asarray(inputs['out_b'], np.float32)[None, None, :]
    return out
